# revision 6
# baseline (speedup 1.0000x reference)
"""KPConv Bass/Trainium2 kernel, v3: dma_gather-based neighbor fetch.

out[m,d] = sum_k ( sum_h infl[m,h,k] * s_feats[idx[m,h],:] ) @ W[k]
infl[m,h,k] = relu(1 - |s_pts[idx[m,h]] - q_pts[m] - kp[k]| / SIGMA)

Sharding: query points M=50000 split 8 ways (6250/core, 49 blocks x 128
points). Support table / weights / kernel_points replicated per core.

Gather: SWDGE descriptor generation (~10ns/desc single queue) is the
kernel's wall; dma_gather (mlp ucode) on 4 SWDGE queues reaches
~5ns/desc aggregate. dma_gather indices are int16, so the support table
is split at row TH=25000 into table_lo/table_hi and each block's edges
are split per-point into lo/hi runs, compacted into per-point-group slot
regions (static capacities), and fetched by 6 dma_gathers/block
(3 lo + 3 hi, 1024/1024/384 idx each) into comb [128, 38*256 u16]
(row = 128 bf16 feats + 6 u16 f32-coord halves + pad; slot i of a chunk
lands at partition i%128, tile i//128).

Blocks hold 128 points in 7 groups (6x21 + 1x2, greedy-balanced so each
group's lo/hi edge counts fit 384/384 or 128/128 slots; output rows are
un-permuted on the host). Step A accumulates per-group PSUM column
windows (group g cols [315g,315g+315)): per tile, matmul(lhsT=feats_t
[128 slots, 128c], rhs=bd_t [128 slots, 315]) with start on the group's
first lo tile and stop on its last hi tile; bd = infl * is_equal(asn,
iota21) maps each slot to its point's columns (dummy slots masked).
Step B per k: matmul(lhsT=wfT[:, k::15], rhs=W[k]) accumulating ->
[128m, 128d] -> DRAM.
"""

import sys

sys.path.insert(0, "/opt/trn_rl_repo")

import numpy as np

# ---------------------------------------------------------------- constants
N_CORES = 8
M_TOTAL = 50000
N_SUP = 50000
H = 32
C = 128
K = 15
SIGMA = 2.0
TH = 25000                           # lo/hi table split row

M_CORE = M_TOTAL // N_CORES          # 6250
P = 128                              # partitions / points per block
NB = (M_CORE + P - 1) // P           # 49 blocks
M_PAD = NB * P                       # 6272
TW = 256                             # u16 cols per table row (512B)

# point groups per block: 2 x 22 + 4 x 21 = 128; per-side slot capacities
GROUPS = [(0, 22), (22, 22), (44, 21), (65, 21), (86, 21), (107, 21)]
GTILES = [3, 3, 3, 3, 3, 3]          # tiles per group per side
GBASE = [0, 3, 6, 9, 12, 15]         # first tile of group within side
TS = 18                              # tiles per side
NT2 = 2 * TS                         # 36 tiles per block
NSIDE = TS * P                       # 2304 slots per side
CHUNKS = [1024, 1024, 256]           # dma_gather sizes per side
# per-tile point windows within a group (drift-bounded by the host's
# alternating big/small member order + load balancing)
WBASE = [0, 6, 13]                   # window base per tile-in-group
WWID = [11, 14, 9]                   # window width per tile-in-group
JW = 14                              # uniform mask width (cols per tile <= 210)

_compiled = None
_perms = None                        # [core][block] -> point permutation


def _build_bass(nb=NB, n_lo=TH, n_hi=N_SUP - TH, compile=True):
    """Build + compile the per-core SPMD Bass program."""
    from contextlib import ExitStack

    import concourse.bacc as bacc
    import concourse.mybir as mybir
    import concourse.tile as tile
    from concourse.library_config import mlp

    f32 = mybir.dt.float32
    bf16 = mybir.dt.bfloat16
    u16 = mybir.dt.uint16
    i16 = mybir.dt.int16
    NB_ = nb

    nc = bacc.Bacc(
        "TRN2",
        target_bir_lowering=False,
        debug=False,
        enable_asserts=False,
        num_devices=N_CORES,
        num_swdge_queues=4,
        dynamic_dma_scratch_size=32768,
    )

    q_blk_d = nc.dram_tensor("q_blk", (NB_, P, NT2 * 3), f32, kind="ExternalInput")
    inds_d = nc.dram_tensor(
        "inds_blk", (NB_, P, 2 * NSIDE // 16), i16, kind="ExternalInput"
    )
    asn_d = nc.dram_tensor("asn_blk", (NB_, P, NT2), f32, kind="ExternalInput")
    tlo_d = nc.dram_tensor("table_lo", (n_lo, TW), u16, kind="ExternalInput")
    thi_d = nc.dram_tensor("table_hi", (n_hi, TW), u16, kind="ExternalInput")
    w_d = nc.dram_tensor("w_ckd", (C, K * C), u16, kind="ExternalInput")
    kp_d = nc.dram_tensor("kp_rep", (P, K * 3), f32, kind="ExternalInput")
    iota_d = nc.dram_tensor("iota21", (P, JW), f32, kind="ExternalInput")
    out_d = nc.dram_tensor("out", (NB_, P, C), f32, kind="ExternalOutput")

    sub = mybir.AluOpType.subtract
    mult = mybir.AluOpType.mult
    iseq = mybir.AluOpType.is_equal

    # static step-A piece table: (tile, col_lo, col_hi, start, stop) with
    # cols global in [0,1920); pieces split at PSUM bank boundaries (480)
    # emitted group-by-group: per (group, PSUM bank), the first piece opens
    # the accumulation chain (start lazily zeroes the whole bank) and the
    # last piece closes it; chains in a bank are strictly sequential.
    # Each tile covers only its drift window [WBASE, WBASE+WWID) of the
    # group's points; rhs_off is the piece's offset into the tile's bd row.
    pieces = []
    for g, (p0, gsz) in enumerate(GROUPS):
        raw = []
        whi = 15 * p0      # written frontier of this group's chain (cols)
        for side in range(2):
            for ti in range(GTILES[g]):
                t = side * TS + GBASE[g] + ti
                if GTILES[g] == 1:
                    wb, ww = 0, gsz
                else:
                    wb, ww = WBASE[ti], min(WWID[ti], gsz - WBASE[ti])
                w0, w1 = 15 * (p0 + wb), 15 * (p0 + wb + ww)
                c0 = w0
                while c0 < w1:
                    # split at bank boundaries AND at the written frontier
                    # (each slice must be uniformly fresh or accumulated)
                    c1 = min(w1, (c0 // 480 + 1) * 480)
                    if c0 < whi < c1:
                        c1 = whi
                    raw.append([t, c0, c1, c0 - w0, False, False])
                    c0 = c1
                whi = max(whi, w1)
        by_bank = {}
        for i, pc in enumerate(raw):
            by_bank.setdefault(pc[1] // 480, []).append(i)
        for idxs in by_bank.values():
            raw[idxs[0]][4] = True    # start
            raw[idxs[-1]][5] = True   # stop
        pieces.extend(tuple(pc) for pc in raw)

    with tile.TileContext(nc) as tc, ExitStack() as ctx:
        const = ctx.enter_context(tc.tile_pool(name="const", bufs=1))
        gio = ctx.enter_context(tc.tile_pool(name="gio", bufs=3))
        io = ctx.enter_context(tc.tile_pool(name="io", bufs=3))
        mid = ctx.enter_context(tc.tile_pool(name="mid", bufs=2))
        psa = ctx.enter_context(tc.tile_pool(name="psa", bufs=1, space="PSUM"))
        psb = ctx.enter_context(tc.tile_pool(name="psb", bufs=2, space="PSUM"))

        nc.gpsimd.load_library(mlp)

        w_sb = const.tile([P, K * C], u16)
        nc.sync.dma_start(w_sb[:], w_d.ap())
        w_view = w_sb[:].bitcast(bf16).rearrange("p (k d) -> p k d", d=C)
        kp_sb = const.tile([P, K * 3], f32)
        nc.sync.dma_start(kp_sb[:], kp_d.ap())
        iota_sb = const.tile([P, JW], f32)
        nc.sync.dma_start(iota_sb[:], iota_d.ap())

        qn = 0
        for B in range(NB_):
            inds = io.tile([P, 2 * NSIDE // 16], i16, tag="inds")
            nc.sync.dma_start(inds[:], inds_d.ap()[B])
            qb = io.tile([P, NT2 * 3], f32, tag="qb")
            nc.sync.dma_start(qb[:], q_blk_d.ap()[B])
            asn = io.tile([P, NT2], f32, tag="asn")
            nc.sync.dma_start(asn[:], asn_d.ap()[B])

            comb = gio.tile([P, NT2 * TW], u16, tag="comb")
            cv = comb[:].rearrange("p (t e) -> p t e", e=TW)
            for side, tab in ((0, tlo_d), (1, thi_d)):
                off16 = side * (NSIDE // 16)
                toff = side * TS
                for ni in CHUNKS:
                    nt = ni // P
                    nc.gpsimd.dma_gather(
                        cv[:, toff : toff + nt, :],
                        tab.ap(),
                        inds[:, off16 : off16 + ni // 16],
                        ni, ni, TW, queue_num=qn % 4,
                    )
                    qn += 1
                    off16 += ni // 16
                    toff += nt

            # influence chain per side (lo chain + lo matmuls overlap the
            # hi-side gathers): delta = s - q; d2 = |delta - kp|^2; infl
            combf = comb[:].bitcast(f32)
            sgv_all = combf.rearrange("p (t x) -> p t x", x=TW // 2)[
                :, :, C // 2 : C // 2 + 3
            ]
            bds = []
            for side in range(2):
                ts0 = side * TS
                delta = mid.tile([P, TS * 3], f32, tag=f"delta{side}")
                nc.vector.tensor_tensor(
                    delta[:].rearrange("p (t j) -> p t j", j=3),
                    sgv_all[:, ts0 : ts0 + TS, :],
                    qb[:, ts0 * 3 : (ts0 + TS) * 3].rearrange(
                        "p (t j) -> p t j", j=3
                    ),
                    op=sub,
                )
                diff = mid.tile([P, TS * K * 3], f32, tag=f"diff{side}")
                nc.vector.tensor_tensor(
                    diff[:].rearrange("p (t k j) -> p t k j", k=K, j=3),
                    delta[:].rearrange("p (t j) -> p t j", j=3)
                    .unsqueeze(2)
                    .broadcast_to([P, TS, K, 3]),
                    kp_sb[:].rearrange("p (k j) -> p k j", j=3)
                    .unsqueeze(1)
                    .broadcast_to([P, TS, K, 3]),
                    op=sub,
                )
                sq = mid.tile([P, TS * K * 3], f32, tag=f"sq{side}")
                nc.scalar.activation(
                    sq[:], diff[:], mybir.ActivationFunctionType.Square
                )
                d2 = mid.tile([P, TS * K], f32, tag=f"d2{side}")
                nc.vector.reduce_sum(
                    out=d2[:],
                    in_=sq[:].rearrange("p (tk j) -> p tk j", j=3),
                    axis=mybir.AxisListType.X,
                )
                dd = mid.tile([P, TS * K], f32, tag=f"dd{side}")
                nc.scalar.sqrt(dd[:], d2[:])
                infl = mid.tile([P, TS * K], mybir.dt.bfloat16, tag=f"infl{side}")
                nc.scalar.activation(
                    infl[:],
                    dd[:],
                    mybir.ActivationFunctionType.Relu,
                    bias=1.0,
                    scale=-1.0 / SIGMA,
                )
                m21 = mid.tile([P, TS * JW], mybir.dt.bfloat16, tag=f"m21{side}")
                nc.vector.tensor_tensor(
                    m21[:].rearrange("p (t j) -> p t j", j=JW),
                    asn[:, ts0 : ts0 + TS].unsqueeze(2).broadcast_to(
                        [P, TS, JW]
                    ),
                    iota_sb[:].unsqueeze(1).broadcast_to([P, TS, JW]),
                    op=iseq,
                )
                bd = mid.tile(
                    [P, TS * JW * K], mybir.dt.bfloat16, tag=f"bd{side}"
                )
                nc.vector.tensor_tensor(
                    bd[:].rearrange("p (t j k) -> p t j k", j=JW, k=K),
                    infl[:].rearrange("p (t k) -> p t k", k=K)
                    .unsqueeze(2)
                    .broadcast_to([P, TS, JW, K]),
                    m21[:].rearrange("p (t j) -> p t j", j=JW)
                    .unsqueeze(3)
                    .broadcast_to([P, TS, JW, K]),
                    op=mult,
                )
                bds.append(bd)

            # step A: accumulate wfT[c, m*15+k] into 4 PSUM banks
            pa = [
                psa.tile([P, 480], f32, tag=f"psA{q}", name=f"psA{q}")
                for q in range(4)
            ]
            for (t, c0, c1, roff, first, last) in pieces:
                bank = c0 // 480
                ts_ = t % TS
                nc.tensor.matmul(
                    pa[bank][:, c0 - 480 * bank : c1 - 480 * bank],
                    lhsT=comb[:, t * TW : t * TW + C].bitcast(
                        mybir.dt.bfloat16
                    ),
                    rhs=bds[t // TS][:, ts_ * (JW * K) + roff :
                                     ts_ * (JW * K) + roff + (c1 - c0)],
                    start=first,
                    stop=last,
                )

            wfT = mid.tile([P, P * K], mybir.dt.bfloat16, tag="wfT")
            for q in range(4):
                nc.scalar.copy(wfT[:, q * 480 : (q + 1) * 480], pa[q][:])

            # step B: accumulate over k
            outp = psb.tile([P, C], f32, tag="outp")
            wview = wfT[:].rearrange("p (m k) -> p k m", k=K)
            for k in range(K):
                nc.tensor.matmul(
                    outp[:],
                    lhsT=wview[:, k, :],
                    rhs=w_view[:, k, :],
                    start=(k == 0),
                    stop=(k == K - 1),
                )
            osb = mid.tile([P, C], f32, tag="osb")
            nc.scalar.copy(osb[:], outp[:])
            nc.sync.dma_start(out_d.ap()[B], osb[:])

    if compile:
        nc.compile()
    return nc


def _to_bf16_u16(x):
    u = np.ascontiguousarray(x, np.float32).view(np.uint32)
    return ((u + 0x7FFF + ((u >> 16) & 1)) >> 16).astype(np.uint16)


def _wrap16(flat):
    """[n] -> [128, n/16] wrapped i16 (idx i at partition i%16, col i//16)."""
    n = flat.shape[0]
    w = flat.astype(np.uint16).reshape(n // 16, 16).T
    return np.tile(w, (8, 1)).view(np.int16)


def _pack_block(idx_blk, q_pts_blk, th):
    """Pack one block's 128 points x H edges into lo/hi slot arrays.

    Returns (ilo[NSIDE], ihi[NSIDE], asn[P, NT2], qc[P, NT2, 3], perm[128])
    where perm[j] = original point index assigned to group position j.
    """
    lo_mask = idx_blk < th
    j_lo = lo_mask.sum(1)

    # greedy balance points into groups (capacity per side per group)
    NG = len(GROUPS)
    order = np.argsort(-j_lo, kind="stable")
    cap = np.array([128 * GTILES[g] for g in range(NG)])
    gsz = np.array([GROUPS[g][1] for g in range(NG)])
    glo = np.zeros(NG, int)
    ghi = np.zeros(NG, int)
    gcount = np.zeros(NG, int)
    members = [[] for _ in range(NG)]
    for p in order:
        jl, jh = j_lo[p], H - j_lo[p]
        best, bestload = -1, None
        for g in range(NG):
            if gcount[g] < gsz[g] and glo[g] + jl <= cap[g] and ghi[g] + jh <= cap[g]:
                load = max(glo[g] + jl, ghi[g] + jh) / cap[g]
                if bestload is None or load < bestload:
                    best, bestload = g, load
        assert best >= 0, "group packing failed"
        members[best].append(p)
        glo[best] += jl
        ghi[best] += jh
        gcount[best] += 1

    # order members big/small alternating by lo-count so the within-group
    # run-position drift stays within the static per-tile windows
    for g in range(NG):
        srt = sorted(members[g], key=lambda p: -j_lo[p])
        inter = []
        a, b = 0, len(srt) - 1
        while a <= b:
            inter.append(srt[a])
            if a != b:
                inter.append(srt[b])
            a, b = a + 1, b - 1
        members[g] = inter

    ilo = np.zeros(NSIDE, np.int64)
    ihi = np.zeros(NSIDE, np.int64)
    asn = np.full((P, NT2), 1.0e6, np.float32)
    qc = np.zeros((P, NT2, 3), np.float32)
    perm = np.zeros(P, np.int64)
    for g in range(NG):
        base = GBASE[g] * P       # slot offset within side
        mem = np.array(members[g], int)
        perm[GROUPS[g][0] : GROUPS[g][0] + len(mem)] = mem
        ntile = GTILES[g]
        for side, (arr, tof) in enumerate(((ilo, 0), (ihi, TS))):
            vals = [
                idx_blk[p][lo_mask[p] if side == 0 else ~lo_mask[p]]
                for p in mem
            ]
            cnts = np.array([len(v) for v in vals])
            flat = np.concatenate(vals) - (0 if side == 0 else th)
            s = base + np.arange(flat.shape[0])
            arr[s] = flat
            li = np.repeat(np.arange(len(mem)), cnts)
            ti = (s - base) // P
            if ntile == 1:
                rel = li
                wid = np.full_like(rel, GROUPS[g][1])
            else:
                rel = li - np.array(WBASE)[ti]
                wid = np.minimum(
                    np.array(WWID)[ti], GROUPS[g][1] - np.array(WBASE)[ti]
                )
            assert (rel >= 0).all() and (rel < wid).all(), (
                f"window violation group {g} side {side}"
            )
            asn[s % P, tof + s // P] = rel
            qc[s % P, tof + s // P] = q_pts_blk[np.repeat(mem, cnts)]
    return ilo, ihi, asn, qc, perm


def _host_prep(q_pts, s_pts, s_feats, neighb_inds, weights, kernel_points):
    """Shard + lay out inputs for the 8 cores."""
    global _perms
    q_pts = np.asarray(q_pts, np.float32)
    s_pts = np.asarray(s_pts, np.float32)
    s_feats = np.asarray(s_feats, np.float32)
    neighb_inds = np.asarray(neighb_inds, np.int64)
    weights = np.asarray(weights, np.float32)
    kernel_points = np.asarray(kernel_points, np.float32)

    table = np.zeros((N_SUP, TW), np.uint16)
    table[:, :C] = _to_bf16_u16(s_feats)
    table[:, C : C + 6] = (
        np.ascontiguousarray(s_pts, "<f4").view(np.uint16).reshape(N_SUP, 6)
    )
    table_lo = np.ascontiguousarray(table[:TH])
    table_hi = np.ascontiguousarray(table[TH:])

    w_ckd = _to_bf16_u16(
        np.ascontiguousarray(weights.transpose(1, 0, 2)).reshape(C, K * C)
    )
    kp_rep = np.broadcast_to(kernel_points.reshape(1, K * 3), (P, K * 3)).copy()
    iota21 = np.broadcast_to(
        np.arange(JW, dtype=np.float32)[None, :], (P, JW)
    ).copy()

    in_maps = []
    _perms = []
    for i in range(N_CORES):
        sl = slice(i * M_CORE, (i + 1) * M_CORE)
        q = np.zeros((M_PAD, 3), np.float32)
        q[:M_CORE] = q_pts[sl]
        idx = np.zeros((M_PAD, H), np.int64)
        idx[:M_CORE] = neighb_inds[sl]
        idx[M_CORE:, H // 2 :] = TH      # pad points: half lo, half hi

        inds_blk = np.zeros((NB, P, 2 * NSIDE // 16), np.int16)
        asn_blk = np.zeros((NB, P, NT2), np.float32)
        q_blk = np.zeros((NB, P, NT2 * 3), np.float32)
        perms = np.zeros((NB, P), np.int64)
        for b in range(NB):
            ilo, ihi, asn, qc, perm = _pack_block(
                idx[b * P : (b + 1) * P], q[b * P : (b + 1) * P], TH
            )
            seq = []
            for side_arr in (ilo, ihi):
                off = 0
                for ni in CHUNKS:
                    seq.append(_wrap16(side_arr[off : off + ni]))
                    off += ni
            inds_blk[b] = np.concatenate(seq, axis=1)
            asn_blk[b] = asn
            q_blk[b] = qc.reshape(P, NT2 * 3)
            perms[b] = perm
        _perms.append(perms)

        in_maps.append(
            {
                "q_blk": q_blk,
                "inds_blk": inds_blk,
                "asn_blk": asn_blk,
                "table_lo": table_lo,
                "table_hi": table_hi,
                "w_ckd": w_ckd,
                "kp_rep": kp_rep,
                "iota21": iota21,
            }
        )
    return in_maps


def kernel(q_pts, s_pts, s_feats, neighb_inds, weights, kernel_points):
    global _compiled
    if _compiled is None:
        _compiled = _build_bass()
    nc = _compiled

    from concourse.bass_utils import run_bass_kernel_spmd

    in_maps = _host_prep(
        q_pts, s_pts, s_feats, neighb_inds, weights, kernel_points
    )
    res = run_bass_kernel_spmd(nc, in_maps, core_ids=list(range(N_CORES)))
    outs = []
    for i, r in enumerate(res.results):
        raw = r["out"].reshape(NB, P, C)
        unperm = np.zeros((NB, P, C), np.float32)
        for b in range(NB):
            unperm[b, _perms[i][b]] = raw[b]
        outs.append(unperm.reshape(M_PAD, C)[:M_CORE])
    return np.concatenate(outs, axis=0).astype(np.float32)


if __name__ == "__main__":
    rng = np.random.default_rng(0)
    ins = {
        "q_pts": rng.standard_normal((M_TOTAL, 3), np.float32),
        "s_pts": rng.standard_normal((N_SUP, 3), np.float32),
        "s_feats": rng.standard_normal((N_SUP, 128), np.float32),
        "neighb_inds": rng.integers(0, N_SUP, (M_TOTAL, H)).astype(np.int32),
        "weights": rng.standard_normal((K, 128, 128), np.float32) * 0.05,
        "kernel_points": rng.standard_normal((K, 3), np.float32),
    }
    out = kernel(**ins)
    print(out.shape, out.dtype)


# revision 7
# speedup vs baseline: 1.1864x; 1.1864x over previous
"""KPConv Bass/Trainium2 kernel, v3: dma_gather-based neighbor fetch.

out[m,d] = sum_k ( sum_h infl[m,h,k] * s_feats[idx[m,h],:] ) @ W[k]
infl[m,h,k] = relu(1 - |s_pts[idx[m,h]] - q_pts[m] - kp[k]| / SIGMA)

Sharding: query points M=50000 split 8 ways (6250/core, 49 blocks x 128
points). Support table / weights / kernel_points replicated per core.

Gather: SWDGE descriptor generation (~10ns/desc single queue) is the
kernel's wall; dma_gather (mlp ucode) on 4 SWDGE queues reaches
~5ns/desc aggregate. dma_gather indices are int16, so the support table
is split at row TH=25000 into table_lo/table_hi and each block's edges
are split per-point into lo/hi runs, compacted into per-point-group slot
regions (static capacities), and fetched by 6 dma_gathers/block
(3 lo + 3 hi, 1024/1024/384 idx each) into comb [128, 38*256 u16]
(row = 128 bf16 feats + 6 u16 f32-coord halves + pad; slot i of a chunk
lands at partition i%128, tile i//128).

Blocks hold 128 points in 7 groups (6x21 + 1x2, greedy-balanced so each
group's lo/hi edge counts fit 384/384 or 128/128 slots; output rows are
un-permuted on the host). Step A accumulates per-group PSUM column
windows (group g cols [315g,315g+315)): per tile, matmul(lhsT=feats_t
[128 slots, 128c], rhs=bd_t [128 slots, 315]) with start on the group's
first lo tile and stop on its last hi tile; bd = infl * is_equal(asn,
iota21) maps each slot to its point's columns (dummy slots masked).
Step B per k: matmul(lhsT=wfT[:, k::15], rhs=W[k]) accumulating ->
[128m, 128d] -> DRAM.
"""

import sys

sys.path.insert(0, "/opt/trn_rl_repo")

import numpy as np

# ---------------------------------------------------------------- constants
N_CORES = 8
M_TOTAL = 50000
N_SUP = 50000
H = 32
C = 128
K = 15
SIGMA = 2.0
TH = 25000                           # lo/hi table split row

M_CORE = M_TOTAL // N_CORES          # 6250
P = 128                              # partitions / points per block
NB = (M_CORE + P - 1) // P           # 49 blocks
M_PAD = NB * P                       # 6272
TW = 256                             # u16 cols per table row (512B)

# point groups per block: 2 x 22 + 4 x 21 = 128; per-side slot capacities
GROUPS = [(0, 22), (22, 22), (44, 21), (65, 21), (86, 21), (107, 21)]
GTILES = [3, 3, 3, 3, 3, 3]          # tiles per group per side
GBASE = [0, 3, 6, 9, 12, 15]         # first tile of group within side
TS = 18                              # tiles per side
NT2 = 2 * TS                         # 36 tiles per block
NSIDE = TS * P                       # 2304 slots per side
CHUNKS = [1024, 1024, 256]           # dma_gather sizes per side
# per-tile point windows within a group (drift-bounded by the host's
# alternating big/small member order + load balancing)
WBASE = [0, 6, 13]                   # window base per tile-in-group
WWID = [11, 14, 9]                   # window width per tile-in-group
JW = 14                              # uniform mask width (cols per tile <= 210)

_compiled = None
_perms = None                        # [core][block] -> point permutation


def _build_bass(nb=NB, n_lo=TH, n_hi=N_SUP - TH, compile=True):
    """Build + compile the per-core SPMD Bass program."""
    from contextlib import ExitStack

    import concourse.bacc as bacc
    import concourse.mybir as mybir
    import concourse.tile as tile
    from concourse.library_config import mlp

    f32 = mybir.dt.float32
    bf16 = mybir.dt.bfloat16
    u16 = mybir.dt.uint16
    i16 = mybir.dt.int16
    NB_ = nb

    nc = bacc.Bacc(
        "TRN2",
        target_bir_lowering=False,
        debug=False,
        enable_asserts=False,
        num_devices=N_CORES,
        num_swdge_queues=4,
        dynamic_dma_scratch_size=32768,
    )

    q_blk_d = nc.dram_tensor("q_blk", (NB_, P, NT2 * 3), f32, kind="ExternalInput")
    inds_d = nc.dram_tensor(
        "inds_blk", (NB_, P, 2 * NSIDE // 16), i16, kind="ExternalInput"
    )
    asn_d = nc.dram_tensor("asn_blk", (NB_, P, NT2), f32, kind="ExternalInput")
    tlo_d = nc.dram_tensor("table_lo", (n_lo, TW), u16, kind="ExternalInput")
    thi_d = nc.dram_tensor("table_hi", (n_hi, TW), u16, kind="ExternalInput")
    w_d = nc.dram_tensor("w_ckd", (C, K * C), u16, kind="ExternalInput")
    kp_d = nc.dram_tensor("kp_rep", (P, K * 3), f32, kind="ExternalInput")
    iota_d = nc.dram_tensor("iota21", (P, JW), f32, kind="ExternalInput")
    out_d = nc.dram_tensor("out", (NB_, P, C), f32, kind="ExternalOutput")

    sub = mybir.AluOpType.subtract
    mult = mybir.AluOpType.mult
    iseq = mybir.AluOpType.is_equal

    # static step-A piece table: (tile, col_lo, col_hi, start, stop) with
    # cols global in [0,1920); pieces split at PSUM bank boundaries (480)
    # emitted group-by-group: per (group, PSUM bank), the first piece opens
    # the accumulation chain (start lazily zeroes the whole bank) and the
    # last piece closes it; chains in a bank are strictly sequential.
    # Each tile covers only its drift window [WBASE, WBASE+WWID) of the
    # group's points; rhs_off is the piece's offset into the tile's bd row.
    pieces = []
    for g, (p0, gsz) in enumerate(GROUPS):
        raw = []
        whi = 15 * p0      # written frontier of this group's chain (cols)
        for side in range(2):
            for ti in range(GTILES[g]):
                t = side * TS + GBASE[g] + ti
                if GTILES[g] == 1:
                    wb, ww = 0, gsz
                else:
                    wb, ww = WBASE[ti], min(WWID[ti], gsz - WBASE[ti])
                w0, w1 = 15 * (p0 + wb), 15 * (p0 + wb + ww)
                c0 = w0
                while c0 < w1:
                    # split at bank boundaries AND at the written frontier
                    # (each slice must be uniformly fresh or accumulated)
                    c1 = min(w1, (c0 // 480 + 1) * 480)
                    if c0 < whi < c1:
                        c1 = whi
                    raw.append([t, c0, c1, c0 - w0, False, False])
                    c0 = c1
                whi = max(whi, w1)
        by_bank = {}
        for i, pc in enumerate(raw):
            by_bank.setdefault(pc[1] // 480, []).append(i)
        for idxs in by_bank.values():
            raw[idxs[0]][4] = True    # start
            raw[idxs[-1]][5] = True   # stop
        pieces.extend(tuple(pc) for pc in raw)

    with tile.TileContext(nc) as tc, ExitStack() as ctx:
        const = ctx.enter_context(tc.tile_pool(name="const", bufs=1))
        gio = ctx.enter_context(tc.tile_pool(name="gio", bufs=4))
        io = ctx.enter_context(tc.tile_pool(name="io", bufs=3))
        mid = ctx.enter_context(tc.tile_pool(name="mid", bufs=2))
        psa = ctx.enter_context(tc.tile_pool(name="psa", bufs=1, space="PSUM"))
        psb = ctx.enter_context(tc.tile_pool(name="psb", bufs=2, space="PSUM"))

        nc.gpsimd.load_library(mlp)

        w_sb = const.tile([P, K * C], u16)
        nc.sync.dma_start(w_sb[:], w_d.ap())
        w_view = w_sb[:].bitcast(bf16).rearrange("p (k d) -> p k d", d=C)
        kp_sb = const.tile([P, K * 3], f32)
        nc.sync.dma_start(kp_sb[:], kp_d.ap())
        iota_sb = const.tile([P, JW], f32)
        nc.sync.dma_start(iota_sb[:], iota_d.ap())

        qn = 0
        for B in range(NB_):
            inds = io.tile([P, 2 * NSIDE // 16], i16, tag="inds")
            nc.sync.dma_start(inds[:], inds_d.ap()[B])
            qb = io.tile([P, NT2 * 3], f32, tag="qb")
            nc.sync.dma_start(qb[:], q_blk_d.ap()[B])
            asn = io.tile([P, NT2], f32, tag="asn")
            nc.sync.dma_start(asn[:], asn_d.ap()[B])

            comb = gio.tile([P, NT2 * TW], u16, tag="comb")
            cv = comb[:].rearrange("p (t e) -> p t e", e=TW)
            for side, tab in ((0, tlo_d), (1, thi_d)):
                off16 = side * (NSIDE // 16)
                toff = side * TS
                for ni in CHUNKS:
                    nt = ni // P
                    nc.gpsimd.dma_gather(
                        cv[:, toff : toff + nt, :],
                        tab.ap(),
                        inds[:, off16 : off16 + ni // 16],
                        ni, ni, TW, queue_num=qn % 4,
                    )
                    qn += 1
                    off16 += ni // 16
                    toff += nt

            # influence chain per side (lo chain + lo matmuls overlap the
            # hi-side gathers): delta = s - q; d2 = |delta - kp|^2; infl
            combf = comb[:].bitcast(f32)
            sgv_all = combf.rearrange("p (t x) -> p t x", x=TW // 2)[
                :, :, C // 2 : C // 2 + 3
            ]
            bds = []
            for side in range(2):
                ts0 = side * TS
                delta = mid.tile([P, TS * 3], f32, tag=f"delta{side}")
                nc.vector.tensor_tensor(
                    delta[:].rearrange("p (t j) -> p t j", j=3),
                    sgv_all[:, ts0 : ts0 + TS, :],
                    qb[:, ts0 * 3 : (ts0 + TS) * 3].rearrange(
                        "p (t j) -> p t j", j=3
                    ),
                    op=sub,
                )
                diff = mid.tile([P, TS * K * 3], f32, tag=f"diff{side}")
                nc.vector.tensor_tensor(
                    diff[:].rearrange("p (t k j) -> p t k j", k=K, j=3),
                    delta[:].rearrange("p (t j) -> p t j", j=3)
                    .unsqueeze(2)
                    .broadcast_to([P, TS, K, 3]),
                    kp_sb[:].rearrange("p (k j) -> p k j", j=3)
                    .unsqueeze(1)
                    .broadcast_to([P, TS, K, 3]),
                    op=sub,
                )
                sq = mid.tile([P, TS * K * 3], f32, tag=f"sq{side}")
                nc.scalar.activation(
                    sq[:], diff[:], mybir.ActivationFunctionType.Square
                )
                d2 = mid.tile([P, TS * K], f32, tag=f"d2{side}")
                nc.vector.reduce_sum(
                    out=d2[:],
                    in_=sq[:].rearrange("p (tk j) -> p tk j", j=3),
                    axis=mybir.AxisListType.X,
                )
                dd = mid.tile([P, TS * K], f32, tag=f"dd{side}")
                nc.scalar.sqrt(dd[:], d2[:])
                infl = mid.tile([P, TS * K], mybir.dt.bfloat16, tag=f"infl{side}")
                nc.scalar.activation(
                    infl[:],
                    dd[:],
                    mybir.ActivationFunctionType.Relu,
                    bias=1.0,
                    scale=-1.0 / SIGMA,
                )
                m21 = mid.tile([P, TS * JW], mybir.dt.bfloat16, tag=f"m21{side}")
                nc.vector.tensor_tensor(
                    m21[:].rearrange("p (t j) -> p t j", j=JW),
                    asn[:, ts0 : ts0 + TS].unsqueeze(2).broadcast_to(
                        [P, TS, JW]
                    ),
                    iota_sb[:].unsqueeze(1).broadcast_to([P, TS, JW]),
                    op=iseq,
                )
                bd = mid.tile(
                    [P, TS * JW * K], mybir.dt.bfloat16, tag=f"bd{side}"
                )
                nc.vector.tensor_tensor(
                    bd[:].rearrange("p (t j k) -> p t j k", j=JW, k=K),
                    infl[:].rearrange("p (t k) -> p t k", k=K)
                    .unsqueeze(2)
                    .broadcast_to([P, TS, JW, K]),
                    m21[:].rearrange("p (t j) -> p t j", j=JW)
                    .unsqueeze(3)
                    .broadcast_to([P, TS, JW, K]),
                    op=mult,
                )
                bds.append(bd)

            # step A: accumulate wfT[c, m*15+k] into 4 PSUM banks
            pa = [
                psa.tile([P, 480], f32, tag=f"psA{q}", name=f"psA{q}")
                for q in range(4)
            ]
            for (t, c0, c1, roff, first, last) in pieces:
                bank = c0 // 480
                ts_ = t % TS
                nc.tensor.matmul(
                    pa[bank][:, c0 - 480 * bank : c1 - 480 * bank],
                    lhsT=comb[:, t * TW : t * TW + C].bitcast(
                        mybir.dt.bfloat16
                    ),
                    rhs=bds[t // TS][:, ts_ * (JW * K) + roff :
                                     ts_ * (JW * K) + roff + (c1 - c0)],
                    start=first,
                    stop=last,
                )

            wfT = mid.tile([P, P * K], mybir.dt.bfloat16, tag="wfT")
            for q in range(4):
                nc.scalar.copy(wfT[:, q * 480 : (q + 1) * 480], pa[q][:])

            # step B: accumulate over k
            outp = psb.tile([P, C], f32, tag="outp")
            wview = wfT[:].rearrange("p (m k) -> p k m", k=K)
            for k in range(K):
                nc.tensor.matmul(
                    outp[:],
                    lhsT=wview[:, k, :],
                    rhs=w_view[:, k, :],
                    start=(k == 0),
                    stop=(k == K - 1),
                )
            osb = mid.tile([P, C], f32, tag="osb")
            nc.scalar.copy(osb[:], outp[:])
            nc.sync.dma_start(out_d.ap()[B], osb[:])

    if compile:
        nc.compile()
    return nc


def _to_bf16_u16(x):
    u = np.ascontiguousarray(x, np.float32).view(np.uint32)
    return ((u + 0x7FFF + ((u >> 16) & 1)) >> 16).astype(np.uint16)


def _wrap16(flat):
    """[n] -> [128, n/16] wrapped i16 (idx i at partition i%16, col i//16)."""
    n = flat.shape[0]
    w = flat.astype(np.uint16).reshape(n // 16, 16).T
    return np.tile(w, (8, 1)).view(np.int16)


def _pack_block(idx_blk, q_pts_blk, th):
    """Pack one block's 128 points x H edges into lo/hi slot arrays.

    Returns (ilo[NSIDE], ihi[NSIDE], asn[P, NT2], qc[P, NT2, 3], perm[128])
    where perm[j] = original point index assigned to group position j.
    """
    lo_mask = idx_blk < th
    j_lo = lo_mask.sum(1)

    # greedy balance points into groups (capacity per side per group)
    NG = len(GROUPS)
    order = np.argsort(-j_lo, kind="stable")
    cap = np.array([128 * GTILES[g] for g in range(NG)])
    gsz = np.array([GROUPS[g][1] for g in range(NG)])
    glo = np.zeros(NG, int)
    ghi = np.zeros(NG, int)
    gcount = np.zeros(NG, int)
    members = [[] for _ in range(NG)]
    for p in order:
        jl, jh = j_lo[p], H - j_lo[p]
        best, bestload = -1, None
        for g in range(NG):
            if gcount[g] < gsz[g] and glo[g] + jl <= cap[g] and ghi[g] + jh <= cap[g]:
                load = max(glo[g] + jl, ghi[g] + jh) / cap[g]
                if bestload is None or load < bestload:
                    best, bestload = g, load
        assert best >= 0, "group packing failed"
        members[best].append(p)
        glo[best] += jl
        ghi[best] += jh
        gcount[best] += 1

    # order members big/small alternating by lo-count so the within-group
    # run-position drift stays within the static per-tile windows
    for g in range(NG):
        srt = sorted(members[g], key=lambda p: -j_lo[p])
        inter = []
        a, b = 0, len(srt) - 1
        while a <= b:
            inter.append(srt[a])
            if a != b:
                inter.append(srt[b])
            a, b = a + 1, b - 1
        members[g] = inter

    ilo = np.zeros(NSIDE, np.int64)
    ihi = np.zeros(NSIDE, np.int64)
    asn = np.full((P, NT2), 1.0e6, np.float32)
    qc = np.zeros((P, NT2, 3), np.float32)
    perm = np.zeros(P, np.int64)
    for g in range(NG):
        base = GBASE[g] * P       # slot offset within side
        mem = np.array(members[g], int)
        perm[GROUPS[g][0] : GROUPS[g][0] + len(mem)] = mem
        ntile = GTILES[g]
        for side, (arr, tof) in enumerate(((ilo, 0), (ihi, TS))):
            vals = [
                idx_blk[p][lo_mask[p] if side == 0 else ~lo_mask[p]]
                for p in mem
            ]
            cnts = np.array([len(v) for v in vals])
            flat = np.concatenate(vals) - (0 if side == 0 else th)
            s = base + np.arange(flat.shape[0])
            arr[s] = flat
            li = np.repeat(np.arange(len(mem)), cnts)
            ti = (s - base) // P
            if ntile == 1:
                rel = li
                wid = np.full_like(rel, GROUPS[g][1])
            else:
                rel = li - np.array(WBASE)[ti]
                wid = np.minimum(
                    np.array(WWID)[ti], GROUPS[g][1] - np.array(WBASE)[ti]
                )
            assert (rel >= 0).all() and (rel < wid).all(), (
                f"window violation group {g} side {side}"
            )
            asn[s % P, tof + s // P] = rel
            qc[s % P, tof + s // P] = q_pts_blk[np.repeat(mem, cnts)]
    return ilo, ihi, asn, qc, perm


def _host_prep(q_pts, s_pts, s_feats, neighb_inds, weights, kernel_points):
    """Shard + lay out inputs for the 8 cores."""
    global _perms
    q_pts = np.asarray(q_pts, np.float32)
    s_pts = np.asarray(s_pts, np.float32)
    s_feats = np.asarray(s_feats, np.float32)
    neighb_inds = np.asarray(neighb_inds, np.int64)
    weights = np.asarray(weights, np.float32)
    kernel_points = np.asarray(kernel_points, np.float32)

    table = np.zeros((N_SUP, TW), np.uint16)
    table[:, :C] = _to_bf16_u16(s_feats)
    table[:, C : C + 6] = (
        np.ascontiguousarray(s_pts, "<f4").view(np.uint16).reshape(N_SUP, 6)
    )
    table_lo = np.ascontiguousarray(table[:TH])
    table_hi = np.ascontiguousarray(table[TH:])

    w_ckd = _to_bf16_u16(
        np.ascontiguousarray(weights.transpose(1, 0, 2)).reshape(C, K * C)
    )
    kp_rep = np.broadcast_to(kernel_points.reshape(1, K * 3), (P, K * 3)).copy()
    iota21 = np.broadcast_to(
        np.arange(JW, dtype=np.float32)[None, :], (P, JW)
    ).copy()

    in_maps = []
    _perms = []
    for i in range(N_CORES):
        sl = slice(i * M_CORE, (i + 1) * M_CORE)
        q = np.zeros((M_PAD, 3), np.float32)
        q[:M_CORE] = q_pts[sl]
        idx = np.zeros((M_PAD, H), np.int64)
        idx[:M_CORE] = neighb_inds[sl]
        idx[M_CORE:, H // 2 :] = TH      # pad points: half lo, half hi

        inds_blk = np.zeros((NB, P, 2 * NSIDE // 16), np.int16)
        asn_blk = np.zeros((NB, P, NT2), np.float32)
        q_blk = np.zeros((NB, P, NT2 * 3), np.float32)
        perms = np.zeros((NB, P), np.int64)
        for b in range(NB):
            ilo, ihi, asn, qc, perm = _pack_block(
                idx[b * P : (b + 1) * P], q[b * P : (b + 1) * P], TH
            )
            seq = []
            for side_arr in (ilo, ihi):
                off = 0
                for ni in CHUNKS:
                    seq.append(_wrap16(side_arr[off : off + ni]))
                    off += ni
            inds_blk[b] = np.concatenate(seq, axis=1)
            asn_blk[b] = asn
            q_blk[b] = qc.reshape(P, NT2 * 3)
            perms[b] = perm
        _perms.append(perms)

        in_maps.append(
            {
                "q_blk": q_blk,
                "inds_blk": inds_blk,
                "asn_blk": asn_blk,
                "table_lo": table_lo,
                "table_hi": table_hi,
                "w_ckd": w_ckd,
                "kp_rep": kp_rep,
                "iota21": iota21,
            }
        )
    return in_maps


def kernel(q_pts, s_pts, s_feats, neighb_inds, weights, kernel_points):
    global _compiled
    if _compiled is None:
        _compiled = _build_bass()
    nc = _compiled

    from concourse.bass_utils import run_bass_kernel_spmd

    in_maps = _host_prep(
        q_pts, s_pts, s_feats, neighb_inds, weights, kernel_points
    )
    res = run_bass_kernel_spmd(nc, in_maps, core_ids=list(range(N_CORES)))
    outs = []
    for i, r in enumerate(res.results):
        raw = r["out"].reshape(NB, P, C)
        unperm = np.zeros((NB, P, C), np.float32)
        for b in range(NB):
            unperm[b, _perms[i][b]] = raw[b]
        outs.append(unperm.reshape(M_PAD, C)[:M_CORE])
    return np.concatenate(outs, axis=0).astype(np.float32)


if __name__ == "__main__":
    rng = np.random.default_rng(0)
    ins = {
        "q_pts": rng.standard_normal((M_TOTAL, 3), np.float32),
        "s_pts": rng.standard_normal((N_SUP, 3), np.float32),
        "s_feats": rng.standard_normal((N_SUP, 128), np.float32),
        "neighb_inds": rng.integers(0, N_SUP, (M_TOTAL, H)).astype(np.int32),
        "weights": rng.standard_normal((K, 128, 128), np.float32) * 0.05,
        "kernel_points": rng.standard_normal((K, 3), np.float32),
    }
    out = kernel(**ins)
    print(out.shape, out.dtype)
